# revision 23
# baseline (speedup 1.0000x reference)
"""GCNII layer on 8 TRN2 NeuronCores (Bass/Tile), v2.

Design (from HW microbenchmarks):
- The kernel is gather-bandwidth-bound: random 256B-row dma_gather peaks
  at ~128GB/s/core only with ~3584-idx calls round-robined across all 4
  SWDGE queues (deeper ring via dynamic_dma_scratch_size=32768).
- Nodes are assigned to 104 chunks of 128 slots by a greedy 4-D balance
  so every (chunk, subrange) edge bucket fits 4x128-edge tiles with no
  uniform-cap padding (1664 tiles/core vs 1960 in v1; pads gather a
  dedicated zero row).
- Edge order is (sub, chunk, src); gather call r,k = rows [k*3584,...)
  of sub r, queue r.  One-hot tiles are built 80% on DVE
  (is_equal+mult tensor_scalar, ~240ns) and 20% on ACT
  (Abs + Relu with per-partition bias/scale, exact one-hot).
- Aggregation: psum[slot,feat] += oh^T @ buf per tile.  Epilogue per
  chunk: ACT scale-copy (0.9*ndst), PE transpose + 0.1*init accumulate,
  ACT copy, PE matmul with M = 0.5(I + W^T), ACT relu, bf16 out.
All float math runs on device; host does integer bucketing/layout only.
"""

import sys

if "/opt/trn_rl_repo" not in sys.path:
    sys.path.insert(0, "/opt/trn_rl_repo")

from contextlib import ExitStack

import ml_dtypes
import numpy as np

N, E, D, NC = 100000, 1600000, 128, 8
NPC = N // NC            # nodes per core: 12500
CH = 104                 # chunks of 128 output slots per core
SLOTS = CH * 128         # 13312
NSUB = 4
SR = 25000               # feature-table rows per subrange
SUBROWS = SR + 1         # +1 zero row
CALL = 3584              # idxs per gather call (28 tiles)
TPC = CALL // 128        # tiles per call: 28
ALPHA, BETA = 0.1, 0.5
ACT_EVERY = 1 << 30      # one-hot builds all on DVE (iota lives in PSUM)

F32 = np.float32
BF16 = ml_dtypes.bfloat16


def _wrap_idx(seq):
    """dma_gather index layout: i -> [i % 16, i // 16], replicated to 128
    partitions (one copy per Q7 core)."""
    blk = seq.reshape(-1, 16).T
    return np.tile(blk, (8, 1))


def _greedy_chunks(d4):
    """Assign NPC nodes (rows of d4 [NPC,4] = per-sub in-degree) to CH
    chunks of <=128 slots, minimizing the max (chunk, sub) bucket."""
    tot = d4.sum(1)
    order = np.argsort(-tot, kind="stable")
    load = np.zeros((CH, 4), np.int64)
    nslots = np.zeros(CH, np.int64)
    chunk_of = np.empty(NPC, np.int64)
    slot_of = np.empty(NPC, np.int64)
    BIG = 1 << 40
    for n in order:
        score = (load + d4[n]).max(1) * 4096 + nslots
        score[nslots >= 128] = BIG
        cc = int(np.argmin(score))
        chunk_of[n] = cc
        slot_of[n] = nslots[cc]
        nslots[cc] += 1
        load[cc] += d4[n]
    return chunk_of, slot_of, load


def _host_prep(features, initial_features, src, dst):
    """Integer-only bucketing/layout -> per-core device arrays + schedule."""
    src = np.ascontiguousarray(src).astype(np.int64, copy=False)
    dst = np.ascontiguousarray(dst).astype(np.int64, copy=False)
    deg = np.bincount(dst, minlength=N)
    degc = np.maximum(deg, 1).astype(F32)
    core_of = dst // NPC
    e_sub_all = src // SR

    tmp = []
    cnts = np.zeros((NC, CH, NSUB), np.int64)
    for c in range(NC):
        em = core_of == c
        e_src = src[em]
        e_loc = dst[em] - c * NPC
        e_sub = e_sub_all[em]
        d4 = np.zeros((NPC, NSUB), np.int64)
        np.add.at(d4, (e_loc, e_sub), 1)
        chunk_of, slot_of, load = _greedy_chunks(d4)
        cnts[c] = load
        tmp.append((e_src, e_loc, e_sub, chunk_of, slot_of))

    TRu = -(-cnts.max(0) // 128)                     # [CH,NSUB] tiles/bucket
    tiles_r = TRu.sum(0)                             # [NSUB]
    G = np.zeros((CH, NSUB), np.int64)               # in-sub tile prefix
    G[1:] = np.cumsum(TRu, 0)[:-1]
    col0 = np.concatenate([[0], np.cumsum(tiles_r)])  # [NSUB+1] col bases
    COLS = int(tiles_r.sum())
    calls_r = [int(-(-tiles_r[r] * 128 // CALL)) for r in range(NSUB)]
    rowbase = np.concatenate([[0], np.cumsum([cr * CALL for cr in calls_r])])
    TOTROWS = int(rowbase[-1])

    sched = dict(TRu=TRu, tiles_r=tiles_r, G=G, col0=col0, COLS=COLS,
                 calls_r=calls_r, rowbase=rowbase, TOTROWS=TOTROWS)

    per_core = []
    for c in range(NC):
        e_src, e_loc, e_sub, chunk_of, slot_of = tmp[c]
        e_chunk = chunk_of[e_loc]
        e_slot = slot_of[e_loc]
        o = np.lexsort((e_src, e_chunk, e_sub))
        es, ec, eb, esl = e_src[o], e_chunk[o], e_sub[o], e_slot[o]
        key = eb * CH + ec
        cnt_flat = np.bincount(key, minlength=NSUB * CH)
        starts = np.zeros(NSUB * CH, np.int64)
        starts[1:] = np.cumsum(cnt_flat)[:-1]
        pos = np.arange(len(es)) - starts[key]
        tile = pos // 128
        part = pos % 128
        col = col0[eb] + G[ec, eb] + tile
        row = rowbase[eb] + (G[ec, eb] + tile) * 128 + part

        seq = np.full(TOTROWS, SR, np.int16)
        seq[row] = (es - eb * SR).astype(np.int16)
        rel = np.full((128, COLS), -1.0, F32)
        rel[part, col] = esl
        dsg = np.ones((128, COLS), F32)
        dsg[part, col] = degc[es]

        dcd = np.ones((128, CH), F32)
        nodes = np.arange(NPC)
        dcd[slot_of, chunk_of] = degc[c * NPC + nodes]
        glob = np.full(SLOTS, -1, np.int64)
        glob[chunk_of * 128 + slot_of] = c * NPC + nodes
        initp = np.zeros((SLOTS, D), F32)
        m = glob >= 0
        initp[m] = initial_features[glob[m]]

        per_core.append(dict(
            eidx=np.ascontiguousarray(_wrap_idx(seq)),
            erel=np.ascontiguousarray(rel),
            edsg=np.ascontiguousarray(dsg),
            dcd=np.ascontiguousarray(dcd),
            initp=np.ascontiguousarray(initp.astype(BF16)),
            glob=glob,
        ))
    return per_core, sched


_BUILD_CACHE = {}


def _build(sched):
    key = sched["TRu"].tobytes()
    if key in _BUILD_CACHE:
        return _BUILD_CACHE[key]
    import concourse.bacc as bacc
    import concourse.mybir as mybir
    import concourse.tile as tile

    f32 = mybir.dt.float32
    bf16 = mybir.dt.bfloat16
    i16 = mybir.dt.int16
    Alu = mybir.AluOpType
    Act = mybir.ActivationFunctionType

    TRu, G, col0 = sched["TRu"], sched["G"], sched["col0"]
    COLS, calls_r, rowbase = sched["COLS"], sched["calls_r"], sched["rowbase"]
    TOTROWS = sched["TOTROWS"]

    nc = bacc.Bacc("TRN2", target_bir_lowering=False, num_swdge_queues=4,
                   dynamic_dma_scratch_size=32768)
    feats = nc.dram_tensor("feats", [NSUB * SUBROWS, D], bf16,
                           kind="ExternalInput")
    wt = nc.dram_tensor("wt", [128, 128], f32, kind="ExternalInput")
    iota = nc.dram_tensor("iota", [128, 128], bf16, kind="ExternalInput")
    ident = nc.dram_tensor("ident", [128, 128], f32, kind="ExternalInput")
    eidx = nc.dram_tensor("eidx", [128, TOTROWS // 16], i16,
                          kind="ExternalInput")
    erel = nc.dram_tensor("erel", [128, COLS], f32, kind="ExternalInput")
    edsg = nc.dram_tensor("edsg", [128, COLS], f32, kind="ExternalInput")
    dcd = nc.dram_tensor("dcd", [128, CH], f32, kind="ExternalInput")
    initp = nc.dram_tensor("initp", [SLOTS, D], bf16, kind="ExternalInput")
    out = nc.dram_tensor("out", [SLOTS, D], bf16, kind="ExternalOutput")

    with tile.TileContext(nc) as tc, ExitStack() as ctx:
        const = ctx.enter_context(tc.tile_pool(name="const", bufs=1))
        gpools = [
            ctx.enter_context(tc.tile_pool(name=f"g{r}", bufs=3))
            for r in range(NSUB)
        ]
        ohpool = ctx.enter_context(tc.tile_pool(name="oh", bufs=16))
        abpool = ctx.enter_context(tc.tile_pool(name="ab", bufs=6))
        epool = ctx.enter_context(tc.tile_pool(name="ep", bufs=8))
        ipool = ctx.enter_context(tc.tile_pool(name="init", bufs=2))
        opool = ctx.enter_context(tc.tile_pool(name="ob", bufs=2))
        ps_agg = ctx.enter_context(tc.tile_pool(name="psagg", bufs=4,
                                                space="PSUM"))
        ps_tr = ctx.enter_context(tc.tile_pool(name="pstr", bufs=1,
                                               space="PSUM"))
        ps_mm = ctx.enter_context(tc.tile_pool(name="psmm", bufs=2,
                                               space="PSUM"))
        ps_io = ctx.enter_context(tc.tile_pool(name="psio", bufs=1,
                                               space="PSUM"))

        iota_sb = const.tile([128, 128], bf16)
        nc.sync.dma_start(out=iota_sb[:], in_=iota[:])
        wt_sb = const.tile([128, 128], f32)
        nc.sync.dma_start(out=wt_sb[:], in_=wt[:])
        id_sb = const.tile([128, 128], f32)
        nc.sync.dma_start(out=id_sb[:], in_=ident[:])
        idx_sb = const.tile([128, TOTROWS // 16], i16)
        nc.sync.dma_start(out=idx_sb[:], in_=eidx[:])
        rel_sb = const.tile([128, COLS], f32)
        nc.sync.dma_start(out=rel_sb[:], in_=erel[:])
        dsg_sb = const.tile([128, COLS], f32)
        nc.sync.dma_start(out=dsg_sb[:], in_=edsg[:])
        dcd_sb = const.tile([128, CH], f32)
        nc.sync.dma_start(out=dcd_sb[:], in_=dcd[:])

        # derived constants
        nsrcf = const.tile([128, COLS], f32)
        nc.scalar.activation(nsrcf[:], dsg_sb[:], Act.Sqrt)
        nc.vector.reciprocal(nsrcf[:], nsrcf[:])
        nsneg = const.tile([128, COLS], f32)
        nc.vector.tensor_scalar(nsneg[:], nsrcf[:], -1.0, None, Alu.mult)
        ndst = const.tile([128, CH], f32)
        nc.scalar.activation(ndst[:], dcd_sb[:], Act.Sqrt)
        nc.vector.reciprocal(ndst[:], ndst[:])
        nc.vector.tensor_scalar(ndst[:], ndst[:], 1.0 - ALPHA, None, Alu.mult)
        id_bf = const.tile([128, 128], bf16)
        nc.vector.tensor_copy(id_bf[:], id_sb[:])
        pid_bf = const.tile([128, 128], bf16)
        nc.vector.tensor_scalar(pid_bf[:], id_sb[:], ALPHA, None, Alu.mult)
        m_tmp = const.tile([128, 128], f32)
        nc.vector.tensor_tensor(m_tmp[:], wt_sb[:], id_sb[:], Alu.add)
        m_bf = const.tile([128, 128], bf16)
        nc.vector.tensor_scalar(m_bf[:], m_tmp[:], BETA, None, Alu.mult)
        iota_f = const.tile([128, 128], f32)
        nc.vector.tensor_copy(iota_f[:], iota_sb[:])
        iota_ps = ps_io.tile([128, 128], f32, space="PSUM")
        nc.tensor.matmul(iota_ps[:], lhsT=id_sb[:], rhs=iota_f[:],
                         start=True, stop=True)

        # gathers: round-robin queues at matched call index
        bufs = [[] for _ in range(NSUB)]
        for k in range(max(calls_r)):
            for r in range(NSUB):
                if k >= calls_r[r]:
                    continue
                buf = gpools[r].tile([128, CALL], bf16)
                bufs[r].append(buf)
                rb = int(rowbase[r]) + k * CALL
                nc.gpsimd.dma_gather(
                    out_ap=buf[:].rearrange("p (t d) -> p t d", d=D),
                    in_ap=feats[r * SUBROWS:(r + 1) * SUBROWS, :],
                    idxs_ap=idx_sb[:, rb // 16:(rb + CALL) // 16],
                    num_idxs=CALL, num_idxs_reg=CALL, elem_size=D,
                    single_packet=False, queue_num=r,
                )

        itiles = {}
        obs = {}

        def emit_agg(c):
            total_mm = int(TRu[c].sum())
            psum = ps_agg.tile([128, 128], f32, space="PSUM")
            mmi = 0
            for r in range(NSUB):
                for t in range(int(TRu[c, r])):
                    col = int(col0[r] + G[c, r] + t)
                    j = int(G[c, r]) + t
                    buf = bufs[r][j // TPC]
                    off = (j % TPC) * 128
                    oh = ohpool.tile([128, 128], bf16)
                    if col % ACT_EVERY == ACT_EVERY - 1:
                        ab = abpool.tile([128, 128], bf16)
                        nc.scalar.activation(
                            ab[:], iota_sb[:], Act.Abs,
                            bias=rel_sb[:, col:col + 1], scale=-1.0)
                        nc.scalar.activation(
                            oh[:], ab[:], Act.Relu,
                            bias=nsrcf[:, col:col + 1],
                            scale=nsneg[:, col:col + 1])
                    else:
                        nc.vector.tensor_scalar(
                            oh[:], iota_ps[:],
                            rel_sb[:, col:col + 1], nsrcf[:, col:col + 1],
                            Alu.is_equal, Alu.mult)
                    nc.tensor.matmul(
                        psum[:], lhsT=oh[:], rhs=buf[:, off:off + 128],
                        start=(mmi == 0), stop=(mmi == total_mm - 1))
                    mmi += 1
            hs = epool.tile([128, 128], bf16)
            nc.scalar.activation(hs[:], psum[:], Act.Copy,
                                 scale=ndst[:, c:c + 1])
            return hs

        def emit_epi(c, hs):
            itile = itiles[c // 8]
            ptr = ps_tr.tile([128, 128], f32, space="PSUM")
            nc.tensor.matmul(ptr[:], lhsT=hs[:], rhs=id_bf[:],
                             start=True, stop=False)
            nc.tensor.matmul(ptr[:],
                             lhsT=itile[:, (c % 8) * 128:(c % 8) * 128 + 128],
                             rhs=pid_bf[:], start=False, stop=True)
            h3t = epool.tile([128, 128], bf16)
            nc.scalar.activation(h3t[:], ptr[:], Act.Copy)
            pmm = ps_mm.tile([128, 128], f32, space="PSUM")
            nc.tensor.matmul(pmm[:], lhsT=h3t[:], rhs=m_bf[:],
                             start=True, stop=True)
            ob = obs[c // 8]
            nc.scalar.activation(ob[:, (c % 8) * 128:(c % 8) * 128 + 128],
                                 pmm[:], Act.Relu)
            if c % 8 == 7 or c == CH - 1:
                c0 = (c // 8) * 8
                nc.sync.dma_start(
                    out=out[c0 * 128:(c + 1) * 128, :]
                    .rearrange("(k p) d -> p k d", p=128),
                    in_=ob[:].rearrange("p (k d) -> p k d", d=D)
                    [:, :c + 1 - c0, :],
                )

        pending = []
        for c in range(CH):
            if c % 8 == 0:
                itile = ipool.tile([128, 8 * 128], bf16)
                hi = min(CH, c + 8)
                nc.sync.dma_start(
                    out=itile[:].rearrange("p (k d) -> p k d", d=D)
                    [:, :hi - c, :],
                    in_=initp[c * 128:hi * 128, :]
                    .rearrange("(k p) d -> p k d", p=128),
                )
                itiles[c // 8] = itile
                obt = opool.tile([128, 8 * 128], bf16)
                obs[c // 8] = obt
            pending.append((c, emit_agg(c)))
            if len(pending) > 2:
                emit_epi(*pending.pop(0))
        for item in pending:
            emit_epi(*item)

    nc.compile()
    _BUILD_CACHE[key] = nc
    return nc


def _install_ntff_shim():
    """antenv.axon_hooks is absent in this image; shim it and wire the real
    NTFF profiling hook via ctypes so trace=True works under axon."""
    import contextlib
    import ctypes
    import types

    try:
        from antenv import axon_hooks  # noqa: F401
        return
    except ImportError:
        pass
    import antenv

    mod = types.ModuleType("antenv.axon_hooks")
    _hook = [None]
    mod.set_axon_ntff_profile_hook = lambda h: _hook.__setitem__(0, h)
    mod.get_axon_ntff_profile_hook = lambda: _hook[0]
    sys.modules["antenv.axon_hooks"] = mod
    antenv.axon_hooks = mod
    try:
        lib = ctypes.CDLL("/opt/axon/libaxon_pjrt.so")
    except OSError:
        return
    if not hasattr(lib, "axon_start_nrt_profile"):
        return
    lib.axon_start_nrt_profile.argtypes = [
        ctypes.POINTER(ctypes.c_int64),
        ctypes.c_size_t,
    ]
    lib.axon_start_nrt_profile.restype = ctypes.c_int64
    lib.axon_stop_nrt_profile.argtypes = [ctypes.c_char_p]
    lib.axon_stop_nrt_profile.restype = ctypes.c_int64

    @contextlib.contextmanager
    def _hook_cm(output_dir, device_ids):
        import jax

        jax.devices()
        if device_ids:
            ids = (ctypes.c_int64 * len(device_ids))(*device_ids)
            rc = lib.axon_start_nrt_profile(ids, len(device_ids))
        else:
            rc = lib.axon_start_nrt_profile(None, 0)
        if rc != 0:
            raise RuntimeError(f"axon_start_nrt_profile rc={rc}")
        try:
            yield
        finally:
            rc = lib.axon_stop_nrt_profile(output_dir.encode())
            if rc != 0:
                print(f"WARNING: axon_stop_nrt_profile rc={rc}", flush=True)

    mod.set_axon_ntff_profile_hook(_hook_cm)


def _run(inputs, trace=False, trace_cores=None):
    from concourse import bass_utils

    if trace:
        _install_ntff_shim()
    features = np.ascontiguousarray(np.asarray(inputs["features"], dtype=F32))
    initial_features = np.ascontiguousarray(
        np.asarray(inputs["initial_features"], dtype=F32)
    )
    W = np.asarray(inputs["W"], dtype=F32)
    src = np.asarray(inputs["src"])
    dst = np.asarray(inputs["dst"])
    per_core, sched = _host_prep(features, initial_features, src, dst)
    nc = _build(sched)
    feats_dev = np.zeros((NSUB * SUBROWS, D), BF16)
    for r in range(NSUB):
        feats_dev[r * SUBROWS:r * SUBROWS + SR] = (
            features[r * SR:(r + 1) * SR].astype(BF16))
    wt_np = np.ascontiguousarray(W.T)
    iota_np = np.ascontiguousarray(
        np.tile(np.arange(128, dtype=F32), (128, 1)).astype(BF16))
    ident_np = np.eye(128, dtype=F32)
    in_maps = []
    for c in range(NC):
        pc = per_core[c]
        in_maps.append(dict(
            feats=feats_dev, wt=wt_np, iota=iota_np, ident=ident_np,
            eidx=pc["eidx"], erel=pc["erel"], edsg=pc["edsg"],
            dcd=pc["dcd"], initp=pc["initp"],
        ))
    res = bass_utils.run_bass_kernel_spmd(
        nc, in_maps, core_ids=list(range(NC)),
        trace=trace, trace_cores=trace_cores,
    )
    result = np.empty((N, D), F32)
    for c in range(NC):
        glob = per_core[c]["glob"]
        oc = np.asarray(res.results[c]["out"], dtype=F32)
        m = glob >= 0
        result[glob[m]] = oc[m]
    return result, res


def kernel(**inputs):
    return _run(inputs, trace=False)[0]
